# revision 49
# baseline (speedup 1.0000x reference)
"""DIN-attention kernel for Trainium2, 8-core SPMD.

Reference computation (per batch b, seq pos l, x = item_seq[b, l]):
    mlp_in = [tgt, x, x-tgt, x*tgt]           (4D = 512)
    h      = relu(mlp_in @ W1 + b1)           (2D = 256)
    score  = h @ W2 + b2                      (1)
    out_b  = sum_l score[l] * x[l] * (l < seq_len[b])

Algebraic restructure (W1 = [A; B; C; Dm] in 128-row blocks):
    z   = x @ (B + C) + (x*tgt) @ Dm + c_b,   c_b = tgt_b @ (A - C) + b1
    out = sum_{l < n_b} (W2.T relu(z) + b2) * x[l]

Device strategy (per core):
  - Batches sorted by seq_len descending; a slot groups 8 global ranks
    [8s, 8s+8), one batch per core, padded to the group max (rounded
    even).  Zero-padded columns contribute exactly 0, so padding is
    safe, and all 8 cores run an identical (SPMD) program while loading
    only ~half the dense bytes.  Slots are then zigzag-interleaved
    (long, short, long, ...) in the token stream so the per-slot fixed
    costs of the DVE reduce spread evenly across tiles instead of
    bunching into a serial tail.
  - Tokens packed host-side into transposed (128=D, T) bf16 arrays
    (X and Y = X*tgt); compute in the hidden-on-partitions layout with
    a fused two-half PSUM tile z[D, 2, n]:
      z[:, h, :] = Wbc_h.T @ X + Wd_h.T @ Y + Cwin_h.T @ IND
    where IND is a host-built 0/1 (32, T) slot-window indicator and
    Cwin packs the c_b bias rows (adds c_b per token on the PE).
  - One ScalarE relu per tile covers both halves ([D, 2, n] AP).
  - Score broadcast to all 128 partitions in one PSUM accumulation:
    P = W2rep0.T @ r0 + W2rep1.T @ r1 (W2rep[k, m] = W2[k] for every m,
    so every output row equals the score row); f32r moving operands.
  - Final per-slot reduce: DVE scalar_tensor_tensor
    acc[:, s] = sum_cols ((P + b2) * X) per slot segment, with aux
    column + add for slot segments that continue across 512-windows.
  - Streaming: chunk sizes ramp in [1280, 2560, 5120] then 6144 to
    hide DMA latency at the head; chunk-0 DMAs are interleaved with
    the constant DMAs in first-use order.
"""

import sys

import numpy as np

for _p in ("/opt/trn_rl_repo",):
    if _p not in sys.path:
        sys.path.insert(0, _p)

import concourse.bacc as bacc
import concourse.bass as bass
import concourse.tile as tile
from concourse import mybir
from concourse.bass_utils import run_bass_kernel_spmd

assert bass  # re-exported for callers

B_FULL = 2048
L_FULL = 200
D = 128
N_CORES = 8
HID = 256  # 2D
TILE_N = 512  # fp32 PSUM bank columns
CHUNK_TARGET = 6144  # tokens per streamed chunk (slot-aligned)
F32 = mybir.dt.float32
F32R = mybir.dt.float32r
BF16 = mybir.dt.bfloat16

HOST_Y_BF16 = True  # ship Y = X*tgt as a host-packed bf16 array
RELU_BF16 = False  # bf16 ACT output is broken on TRN2 HW (probe E); use f32r
XB_BF16 = True  # ship X itself in bf16 (halves X DMA; bf16 h-matmuls)
RAMP_CAPS = [1536, 2560, 5120]  # chunk-size ramp-in (hide pipeline fill)
TAIL_CAP = 1024  # last-chunk cap (fast drain)
CHUNK_BOUNDS = None  # explicit slot-index chunk boundaries (overrides caps)
STREAM_BUFS = 2  # chunk-level double buffering
RB_BUFS = 3  # relu tile buffering
DB_BUFS = 2  # dump tile buffering
PB_BUFS = 3  # pbc PSUM buffering (banks)
B2VAL = [0.0]  # b2 constant, set by build_all before tracing
SEG_COMBINE_MAX = 0  # tiles with <= this many segs use the u-combine score path (0=off)
ACT_BIAS_MIN_LEN = 10**9  # slots at least this long get bias via ACT relu (skip ind-mm)


def _plan(seq_len):
    """Slot plan shared by all cores (SPMD: identical program).

    Grouping (which 8 batches share a slot) is by descending seq_len so
    padding to the per-slot max is cheap.  The slot ORDER in the token
    stream is then zigzag-interleaved (long, short, long, short, ...) so
    the many tiny slots - each costing a fixed PSUM-access init on the
    DVE reduce - spread evenly across tiles instead of bunching in the
    final tiles and serializing into a tail.
    """
    n = np.clip(np.asarray(seq_len).astype(np.int64), 0, L_FULL)
    order = np.argsort(-n, kind="stable")  # descending
    n_sorted = n[order]
    rank_lens = []
    for s in range(B_FULL // N_CORES):
        m = int(n_sorted[N_CORES * s])  # max of ranks [8s, 8s+8)
        if m <= 0:
            break
        rank_lens.append(m + (m & 1))  # round up to even
    S = len(rank_lens)
    perm = []  # stream position -> sorted rank
    i, j = 0, S - 1
    while i <= j:
        perm.append(i)
        if i != j:
            perm.append(j)
        i += 1
        j -= 1
    slot_lens = [rank_lens[p] for p in perm]
    # batch for (stream slot p, core k)
    slot_batches = np.array(
        [[int(order[N_CORES * perm[p] + k]) for k in range(N_CORES)] for p in range(S)],
        dtype=np.int64,
    )
    offs = np.zeros(S + 1, dtype=np.int64)
    offs[1:] = np.cumsum(slot_lens)
    T = int(offs[-1])

    # chunks: contiguous slot ranges, ramped-in sizes, small tail chunk.
    chunks = []  # (slot_a, slot_b, tok_off, tok_len)
    if CHUNK_BOUNDS is not None:
        bounds = [b for b in CHUNK_BOUNDS if 0 < b < S] + [S]
        sa = 0
        for sb in bounds:
            if sb <= sa:
                continue
            chunks.append((sa, sb, int(offs[sa]), int(offs[sb] - offs[sa])))
            sa = sb
        return n, slot_batches, slot_lens, offs, T, chunks
    sa = 0
    while sa < S:
        if len(chunks) < len(RAMP_CAPS):
            cap = RAMP_CAPS[len(chunks)]
        else:
            cap = CHUNK_TARGET
        rem = int(offs[S] - offs[sa])
        # leave room for a small drain chunk at the end
        if rem > cap and rem - cap < TAIL_CAP:
            cap = rem - TAIL_CAP
        sb = sa
        while sb < S and offs[sb + 1] - offs[sa] <= cap:
            sb += 1
        if sb == sa:
            sb = sa + 1
        chunks.append((sa, sb, int(offs[sa]), int(offs[sb] - offs[sa])))
        sa = sb
    return n, slot_batches, slot_lens, offs, T, chunks


def _build_program(slot_lens, offs, T, chunks):
    S = len(slot_lens)
    NW = (S + 31) // 32  # 32-slot bias windows
    nc = bacc.Bacc("TRN2", target_bir_lowering=False, debug=False)

    RDT = BF16 if RELU_BF16 else F32R
    YDT = BF16 if HOST_Y_BF16 else F32
    XDT = BF16 if XB_BF16 else F32R

    xt_d = nc.dram_tensor("xt", [D, T], XDT, kind="ExternalInput")
    ind_d = nc.dram_tensor("ind", [32, T], BF16, kind="ExternalInput")
    if HOST_Y_BF16:
        yb_d = nc.dram_tensor("yb", [D, T], BF16, kind="ExternalInput")
    else:
        tgt_d = nc.dram_tensor("tgt", [D, S], F32, kind="ExternalInput")
    cbw_d = nc.dram_tensor("cbw", [32, NW * HID], BF16, kind="ExternalInput")
    wbc_d = nc.dram_tensor("wbc", [D, HID], XDT, kind="ExternalInput")
    wd_d = nc.dram_tensor("wd", [D, HID], YDT, kind="ExternalInput")
    w2r_d = nc.dram_tensor("w2r", [D, HID], RDT, kind="ExternalInput")
    dcol_d = nc.dram_tensor("dcol", [D, 2], F32, kind="ExternalInput")
    cb_d = nc.dram_tensor("cb", [D, 2 * S], F32, kind="ExternalInput")
    out_d = nc.dram_tensor("out_t", [D, 256], F32, kind="ExternalOutput")

    cmax = max(c[3] for c in chunks)

    with tile.TileContext(nc) as tc:
        with (
            tc.tile_pool(name="const", bufs=1) as cpool,
            tc.tile_pool(name="xst", bufs=STREAM_BUFS) as xpool,
            tc.tile_pool(name="yst", bufs=STREAM_BUFS) as ypool,
            tc.tile_pool(name="ist", bufs=STREAM_BUFS) as ipool,
            tc.tile_pool(name="rst", bufs=RB_BUFS) as rpool,
            tc.tile_pool(name="dst", bufs=DB_BUFS) as dpool,
            tc.tile_pool(name="ust", bufs=2) as upool,
            tc.tile_pool(name="ps", bufs=2, space="PSUM") as pspool,
            tc.tile_pool(name="pp", bufs=PB_BUFS, space="PSUM") as ppool,
        ):
            wbc = cpool.tile([D, HID], XDT, tag="wbc")
            wd = cpool.tile([D, HID], YDT, tag="wd")
            w2r = cpool.tile([D, HID], RDT, tag="w2r")
            cbw = cpool.tile([32, NW * HID], BF16, tag="cbw")
            dcol = cpool.tile([D, 2], F32, tag="dcol")
            cb = cpool.tile([D, 2 * S], F32, tag="cb")
            acc = cpool.tile([D, 256], F32, tag="acc")
            aux = cpool.tile([D, 2], F32, tag="aux")

            if not HOST_Y_BF16:
                tgt = cpool.tile([D, S], F32, tag="tgt")
            nc.vector.memset(acc[:], 0.0)

            for ci, (sa, sb, toff, tlen) in enumerate(chunks):
                x = xpool.tile([D, cmax], XDT, tag="x")
                y = ypool.tile([D, cmax], YDT, tag="y")
                indt = ipool.tile([32, cmax], BF16, tag="ind")
                # head: interleave const DMAs with chunk-0 streams in
                # first-use order so the first matmul starts ASAP
                if ci == 0:
                    nc.sync.dma_start(out=wbc[:], in_=wbc_d[:])
                nc.sync.dma_start(out=x[:, :tlen], in_=xt_d[:, toff : toff + tlen])
                if ci == 0:
                    nc.sync.dma_start(out=wd[:], in_=wd_d[:])
                if HOST_Y_BF16:
                    nc.sync.dma_start(
                        out=y[:, :tlen], in_=yb_d[:, toff : toff + tlen]
                    )
                if ci == 0:
                    nc.sync.dma_start(out=cbw[:], in_=cbw_d[:])
                    if ACT_BIAS_MIN_LEN <= L_FULL:
                        nc.sync.dma_start(out=cb[:], in_=cb_d[:])
                nc.sync.dma_start(out=indt[:, :tlen], in_=ind_d[:, toff : toff + tlen])
                if ci == 0:
                    nc.sync.dma_start(out=w2r[:], in_=w2r_d[:])
                    if SEG_COMBINE_MAX > 0:
                        nc.sync.dma_start(out=dcol[:], in_=dcol_d[:])
                    if not HOST_Y_BF16:
                        nc.sync.dma_start(out=tgt[:], in_=tgt_d[:])

                if not HOST_Y_BF16:
                    # Y = X * tgt_b  (per-slot columns, per-partition scalar)
                    for s in range(sa, sb):
                        a = int(offs[s] - toff)
                        b = int(offs[s + 1] - toff)
                        nc.gpsimd.tensor_scalar_mul(
                            y[:, a:b], x[:, a:b].bitcast(F32), tgt[:, s : s + 1]
                        )

                WIN = 2 * TILE_N  # pbc window: 2 PSUM banks, fewer stt segs
                for w0 in range(0, tlen, WIN):
                    w1 = min(tlen, w0 + WIN)
                    pbc = ppool.tile([D, WIN], F32, tag="pbc")
                    for c0 in range(w0, w1, TILE_N):
                        c1 = min(w1, c0 + TILE_N)
                        n = c1 - c0
                        # slot segments covered by this sub-tile
                        segs = []
                        for s in range(sa, sb):
                            a = max(int(offs[s] - toff), c0)
                            b = min(int(offs[s + 1] - toff), c1)
                            if a < b:
                                segs.append((s, a, b))

                        # long slots take their bias inside the ACT relu
                        # (per-partition bias AP); the rest via ind matmul
                        bias_segs = [
                            t for t in segs if slot_lens[t[0]] >= ACT_BIAS_MIN_LEN
                        ]
                        ind_segs = [
                            t for t in segs if slot_lens[t[0]] < ACT_BIAS_MIN_LEN
                        ]
                        # contiguous runs of ind segs, split at 32-slot windows
                        mms = []  # (window, a, b)
                        runs = []
                        for t in ind_segs:
                            if runs and runs[-1][-1][2] == t[1]:
                                runs[-1].append(t)
                            else:
                                runs.append([t])
                        for run in runs:
                            parts = {}
                            for s, a, b in run:
                                w = s // 32
                                if w in parts:
                                    parts[w] = (
                                        min(parts[w][0], a),
                                        max(parts[w][1], b),
                                    )
                                else:
                                    parts[w] = (a, b)
                            for w, (a, b) in sorted(parts.items()):
                                mms.append((w, a, b))

                        # fused z tile: [D, 2, TILE_N] = both hidden halves in
                        # one 2-bank PSUM tile
                        z = pspool.tile([D, 2, TILE_N], F32, tag="z")
                        for h in (0, 1):
                            hs = slice(h * D, h * D + D)
                            zh = z[:, h, :n]
                            nc.tensor.matmul(
                                zh,
                                wbc[:, hs],
                                x[:, c0:c1],
                                start=True,
                                stop=False,
                            )
                            nc.tensor.matmul(
                                zh,
                                wd[:, hs],
                                y[:, c0:c1],
                                start=False,
                                stop=not mms,
                            )
                            for mi, (w, a, b) in enumerate(mms):
                                nc.tensor.matmul(
                                    z[:, h, a - c0 : b - c0],
                                    cbw[
                                        :, w * HID + h * D : w * HID + h * D + D
                                    ],
                                    indt[:, a:b],
                                    start=False,
                                    stop=(mi == len(mms) - 1),
                                )

                        r = rpool.tile([D, 2, TILE_N], RDT, tag="r")
                        for run in runs:
                            a, b = run[0][1], run[-1][2]
                            nc.scalar.activation(
                                r[:, :, a - c0 : b - c0],
                                z[:, :, a - c0 : b - c0],
                                mybir.ActivationFunctionType.Relu,
                            )
                        for s, a, b in bias_segs:
                            for h in (0, 1):
                                nc.scalar.activation(
                                    r[:, h, a - c0 : b - c0],
                                    z[:, h, a - c0 : b - c0],
                                    mybir.ActivationFunctionType.Relu,
                                    bias=cb[:, 2 * s + h : 2 * s + h + 1],
                                )

                        # P[:, t] = score(t) on every partition
                        if RELU_BF16:
                            w2r0, w2r1 = w2r[:, 0:D], w2r[:, D:HID]
                            rr0, rr1 = r[:, 0, :n], r[:, 1, :n]
                        else:
                            w2r0 = w2r[:, 0:D].bitcast(F32R)
                            w2r1 = w2r[:, D:HID].bitcast(F32R)
                            rr0 = r[:, 0, :n].bitcast(F32R)
                            rr1 = r[:, 1, :n].bitcast(F32R)
                        pw = pbc[:, c0 - w0 : c0 - w0 + n]
                        if 0 < len(segs) <= SEG_COMBINE_MAX:
                            # u = r0 + d*r1 on DVE, then a single score matmul
                            u = upool.tile([D, TILE_N], F32, tag="u")
                            nc.vector.scalar_tensor_tensor(
                                out=u[:, :n],
                                in0=r[:, 1, :n].bitcast(F32),
                                scalar=dcol[:, 0:1],
                                in1=r[:, 0, :n].bitcast(F32),
                                op0=mybir.AluOpType.mult,
                                op1=mybir.AluOpType.add,
                            )
                            nc.tensor.matmul(
                                pw, w2r0, u[:, :n].bitcast(F32R),
                                start=True, stop=True,
                            )
                        else:
                            nc.tensor.matmul(pw, w2r0, rr0, start=True, stop=False)
                            nc.tensor.matmul(pw, w2r1, rr1, start=False, stop=True)

                    # per-slot fused multiply-reduce over the whole window
                    wsegs = []
                    for s in range(sa, sb):
                        a = max(int(offs[s] - toff), w0)
                        b = min(int(offs[s + 1] - toff), w1)
                        if a < b:
                            wsegs.append((s, a, b))
                    dump = dpool.tile([D, WIN], F32, tag="dump")
                    for s, a, b in wsegs:
                        first = a == int(offs[s] - toff)
                        tgt_col = acc[:, s : s + 1] if first else aux[:, 0:1]
                        nc.vector.scalar_tensor_tensor(
                            out=dump[:, a - w0 : b - w0],
                            in0=pbc[:, a - w0 : b - w0],
                            scalar=B2VAL[0],
                            in1=(
                                x[:, a:b]
                                if XB_BF16
                                else x[:, a:b].bitcast(F32)
                            ),
                            op0=mybir.AluOpType.add,
                            op1=mybir.AluOpType.mult,
                            accum_out=tgt_col,
                        )
                        if not first:
                            nc.vector.tensor_add(
                                acc[:, s : s + 1],
                                acc[:, s : s + 1],
                                aux[:, 0:1],
                            )

            nc.sync.dma_start(out=out_d[:], in_=acc[:])
    nc.compile()
    return nc


def _pack_core(item_seq, target, cmat, nvec, slot_batches, slot_lens, offs, T, core):
    S = len(slot_lens)
    NW = (S + 31) // 32
    x_nat = np.zeros((T, D), dtype=np.float32)
    y_nat = np.zeros((T, D), dtype=np.float32) if HOST_Y_BF16 else None
    from ml_dtypes import bfloat16

    ind = np.zeros((32, T), dtype=bfloat16)
    tgt = np.zeros((D, S), dtype=np.float32)
    cbw = np.zeros((32, NW * HID), dtype=bfloat16)
    cb = np.zeros((D, 2 * S), dtype=np.float32)
    for s in range(S):
        b = int(slot_batches[s, core])
        o = int(offs[s])
        nb = int(nvec[b])
        if nb > 0:
            x_nat[o : o + nb] = item_seq[b, :nb]
            if y_nat is not None:
                y_nat[o : o + nb] = item_seq[b, :nb] * target[b]
        ind[s % 32, o : o + slot_lens[s]] = 1.0
        tgt[:, s] = target[b]
        cbw[s % 32, (s // 32) * HID : (s // 32 + 1) * HID] = cmat[b]
        cb[:, 2 * s] = cmat[b, 0:D]
        cb[:, 2 * s + 1] = cmat[b, D:HID]
    xt = np.ascontiguousarray(x_nat.T)
    if XB_BF16:
        from ml_dtypes import bfloat16

        xt = xt.astype(bfloat16)
    m = {"xt": xt, "ind": ind, "cbw": cbw, "cb": cb}
    if HOST_Y_BF16:
        from ml_dtypes import bfloat16

        m["yb"] = np.ascontiguousarray(y_nat.T).astype(bfloat16)
    else:
        m["tgt"] = tgt
    return m


def build_all(target, item_seq, seq_len, W1, b1, W2, b2):
    """Build (nc, in_maps, assemble) without running — used by kernel()
    and by test harnesses that want to run/profile the program."""
    target = np.asarray(target, dtype=np.float32)
    item_seq = np.asarray(item_seq, dtype=np.float32)
    W1 = np.asarray(W1, dtype=np.float32)
    b1 = np.asarray(b1, dtype=np.float32)
    W2 = np.asarray(W2, dtype=np.float32)
    b2 = np.asarray(b2, dtype=np.float32)

    nvec, slot_batches, slot_lens, offs, T, chunks = _plan(seq_len)
    S = len(slot_lens)

    W1a, W1b = W1[0:D], W1[D : 2 * D]
    W1c, W1d = W1[2 * D : 3 * D], W1[3 * D : 4 * D]
    wbc = np.ascontiguousarray(W1b + W1c)
    wd = np.ascontiguousarray(W1d)
    cmat = (target @ (W1a - W1c) + b1).astype(np.float32)  # (B, 256)
    # permute hidden units by |W2| desc so pairing (k, k+128) always has
    # |W2[half1]| <= |W2[half0]|; then score = W2h0 . (r0 + d*r1), |d|<=1
    w2flat = W2[:, 0]
    hperm = np.argsort(-np.abs(w2flat), kind="stable")
    wbc = np.ascontiguousarray(wbc[:, hperm])
    wd = np.ascontiguousarray(wd[:, hperm])
    cmat = np.ascontiguousarray(cmat[:, hperm])
    w2p = w2flat[hperm]
    w2a, w2b = w2p[0:D], w2p[D:HID]
    dvals = np.where(w2a != 0.0, w2b / np.where(w2a == 0.0, 1.0, w2a), 0.0)
    dcol = np.zeros((D, 2), dtype=np.float32)
    dcol[:, 0] = dvals
    w2r = np.empty((D, HID), dtype=np.float32)
    w2r[:, 0:D] = np.repeat(w2a[:, None], D, axis=1)  # [k, m] = W2h0[k]
    w2r[:, D:HID] = np.repeat(w2b[:, None], D, axis=1)
    B2VAL[0] = float(np.asarray(b2).reshape(-1)[0])

    if HOST_Y_BF16 or RELU_BF16:
        from ml_dtypes import bfloat16
    if HOST_Y_BF16:
        wd = wd.astype(bfloat16)
    if XB_BF16:
        wbc = wbc.astype(bfloat16)
    if RELU_BF16:
        w2r = w2r.astype(bfloat16)

    nc = _build_program(slot_lens, offs, T, chunks)

    shared = {"wbc": wbc, "wd": wd, "w2r": w2r, "dcol": dcol}
    in_maps = []
    for k in range(N_CORES):
        m = _pack_core(
            item_seq, target, cmat, nvec, slot_batches, slot_lens, offs, T, k
        )
        m.update(shared)
        in_maps.append(m)

    def assemble(results):
        out = np.zeros((B_FULL, D), dtype=np.float32)
        for k in range(N_CORES):
            ot = np.asarray(results[k]["out_t"])  # (128, 256)
            for s in range(S):
                out[int(slot_batches[s, k])] = ot[:, s]
        return out

    return nc, in_maps, assemble


def kernel(target, item_seq, seq_len, W1, b1, W2, b2):
    nc, in_maps, assemble = build_all(target, item_seq, seq_len, W1, b1, W2, b2)
    res = run_bass_kernel_spmd(nc, in_maps, list(range(N_CORES)))
    results = res.results if hasattr(res, "results") else res
    return assemble(results)

